# revision 27
# baseline (speedup 1.0000x reference)
"""Trainium2 Bass kernel for the gated-MLP-over-ring-buffer problem.

Reference computation (B=512, M=128, V=256, H=256, IN = M*V = 32768):
    mem    = roll(memory, 1, axis=1); mem[:, 0, :] = x        # [B, M, V]
    flat   = mem.reshape(B, IN)                                # [B, 32768]
    h      = tanh(flat @ W1 + b1) * sigmoid(flat @ Wg + bg)    # [B, 256]
    logits = h @ W2 + b2                                       # [B, 256]

Strategy (8 NeuronCores, one trn2 device), default mode "rs":
  - Contraction-shard the two big GEMMs: core c owns k-rows
    [4096c, 4096(c+1)) of W1/Wg and the matching slab of flat.T
    (host-prepared, transposed + packed so every DMA line is >=2KB
    contiguous per partition, bf16).
  - Each core computes partial P1.T / Pg.T = W.T @ flat.T  -> [2H, B]
    accumulated over its 32 k-chunks in PSUM (bf16 operands, f32 acc).
  - Partials are downcast to bf16 with b/8 bias pre-folded and cross-
    core reduced with a single ReduceScatter(add) over batch, so core c
    ends up with the fully reduced batch columns [64c, 64c+64).  A tiny
    dummy collective on 4-core subgroups issued at program start pulls
    the one-time CC bootstrap barrier (~46us floor: CC-core boot 21.6us
    + ~24us barrier) off the critical path; the real ReduceScatter then
    starts right after the ~11us stream setup + ~8us dummy op.
  - Each core applies tanh/sigmoid gating (bias-free ACTIVATEs over
    fused [128,2,64] tiles) and the small bf16 W2 GEMM for its batch
    chunk, writing logits.T [V, 64] in bf16.
  - Host assembles/transposes/upcasts the 8 chunks back to [B, V].

An experimental mode "rdma" (KERNEL_CC=rdma) hand-rolls the exchange
with remote_dma_broadcast over XOR-relative physical destinations
(probed driver map phys = [0,1,2,3,6,7,4,5]); it is kept for reference
but the CC-core bootstrap conflicts with the gpsimd remote-dma ucode
(descriptor corruption / multi-ms delivery stalls without fabric init),
so "rs" is the production path.
"""

import os

import numpy as np

import concourse.bacc as bacc
import concourse.bass as bass
import concourse.mybir as mybir
import concourse.tile as tile
from concourse import bass_utils

B, M, V, H = 512, 128, 256, 256
IN = M * V              # 32768
NCORES = 8
KC = IN // NCORES       # 4096 contraction rows per core
NKG = 8                 # DMA k-groups per core
KB_PER_G = KC // (NKG * 128)  # 4 k-chunks of 128 per group
BCHUNK = B // NCORES    # 64 batch columns per core

F32 = mybir.dt.float32
BF16 = mybir.dt.bfloat16
AF = mybir.ActivationFunctionType

CC = os.environ.get("KERNEL_CC", "rs")  # rs | rdma

_CACHE = {}


def _stage2(nc, s2pool, psum2, s2a, s2b, bt, w2t, outT):
    """Gate + W2 for the local batch chunk.

    s2a: SBUF AP [128, 2, BCHUNK] = [p1_h0, p1_h1] row-blocks (bias
    pre-folded before the reduction); s2b likewise [pg_h0, pg_h1].
    """
    th = s2pool.tile([128, 2, BCHUNK], F32, tag="th", name="th")
    nc.scalar.activation(th[:], s2a[:], AF.Tanh)
    sg = s2pool.tile([128, 2, BCHUNK], F32, tag="sg", name="sg")
    nc.scalar.activation(sg[:], s2b[:], AF.Sigmoid)
    ht = s2pool.tile([128, 2, BCHUNK], BF16, tag="ht", name="ht")
    nc.vector.tensor_mul(ht[:], th[:], sg[:])

    for v in range(2):
        ps = psum2.tile([128, BCHUNK], F32, tag=f"acc{v}", name=f"ps2_{v}")
        for i in range(2):
            nc.tensor.matmul(
                ps[:],
                w2t[:, i, bass.ts(v, 128)],
                ht[:, i, :],
                start=(i == 0),
                stop=(i == 1),
            )
        ot = s2pool.tile([128, BCHUNK], BF16, tag=f"ot{v}", name=f"ot{v}")
        nc.vector.tensor_scalar_add(ot[:], ps[:], bt[:, 4 + v : 5 + v])
        nc.sync.dma_start(out=outT[bass.ts(v, 128), :], in_=ot[:])


def _build(cc=CC):
    nc = bacc.Bacc(
        "TRN2",
        target_bir_lowering=False,
        debug=False,
        enable_asserts=False,
        num_devices=NCORES,
    )

    GROUPS = [list(range(NCORES))]

    # Per-core external inputs (host pre-packed so each DMA moves long
    # contiguous lines per partition: memT 4KB, weights 2KB).
    memT = nc.dram_tensor("memT", [NKG, 128, KB_PER_G * B], BF16, kind="ExternalInput")
    w1 = nc.dram_tensor("w1", [NKG, 128, KB_PER_G * H], BF16, kind="ExternalInput")
    wg = nc.dram_tensor("wg", [NKG, 128, KB_PER_G * H], BF16, kind="ExternalInput")
    # W2 pre-transposed to [p, c, v] bf16 on host.
    w2 = nc.dram_tensor("w2", [128, 2, V], BF16, kind="ExternalInput")
    # packed biases: cols = [b1_lo/8, b1_hi/8, bg_lo/8, bg_hi/8, b2_lo, b2_hi]
    bpk = nc.dram_tensor("bpk", [128, 6], F32, kind="ExternalInput")
    outT = nc.dram_tensor("outT", [V, BCHUNK], BF16, kind="ExternalOutput")

    with tile.TileContext(nc) as tc:
        with (
            tc.tile_pool(name="xg", bufs=4) as xpool,
            tc.tile_pool(name="wt", bufs=4) as wpool,
            tc.tile_pool(name="s2", bufs=1) as s2pool,
            tc.tile_pool(name="psum1", bufs=1, space="PSUM") as psum1,
            tc.tile_pool(name="dram", bufs=1, space="DRAM") as dpool,
        ):
            if cc == "rs":
                # Dummy collective pulls the CC bootstrap barrier early.
                dg = os.environ.get("KERNEL_DUMMY", "quad")
                dgroups = [[0, 1, 2, 3], [4, 5, 6, 7]]
                if dg == "self":
                    dgroups = [[c] for c in range(NCORES)]
                dd_in = dpool.tile([4, 64], BF16, tag="ddin", name="ddin")
                dd_out = dpool.tile([64], BF16, tag="ddout", name="ddout")
                dseed = s2pool.tile([4, 64], BF16, tag="dseed")
                nc.gpsimd.memset(dseed[:], 0.0)
                nc.gpsimd.dma_start(out=dd_in[:], in_=dseed[:])
                din = dd_in[0:1, :] if dg == "self" else dd_in[:]
                nc.gpsimd.collective_compute(
                    "ReduceScatter",
                    mybir.AluOpType.add,
                    replica_groups=dgroups,
                    ins=[din.opt()],
                    outs=[dd_out[:].opt()],
                )

            # Pre-warm the Tanh/Sigmoid activation tables off the critical
            # path (the first use of each table pays a ~1.3us load).
            warm = s2pool.tile([128, 1], F32, tag="warm")
            nc.gpsimd.memset(warm[:], 0.0)
            warm2 = s2pool.tile([128, 1], F32, tag="warm2")
            nc.scalar.activation(warm2[:], warm[:], AF.Tanh)
            nc.scalar.activation(warm[:], warm2[:], AF.Sigmoid)

            # Pre-warm the PE HAM clock gate with dummy matmuls while the
            # first input DMAs are in flight.
            wsrc = s2pool.tile([128, B], BF16, tag="wsrc")
            nc.gpsimd.memset(wsrc[:], 0.0)
            wps = psum1.tile([128, B], F32, tag="acc7", name="wps")
            for i in range(20):
                nc.tensor.matmul(
                    wps[:], wsrc[:, 0:128], wsrc[:], start=(i == 0), stop=(i == 19)
                )

            # Stage-2 constants on the (otherwise idle) gpsimd SWDGE queue.
            bt = s2pool.tile([128, 6], F32, tag="bias")
            nc.gpsimd.dma_start(out=bt[:], in_=bpk[:, :])
            w2t = s2pool.tile([128, 2, V], BF16, tag="w2")
            nc.gpsimd.dma_start(out=w2t[:], in_=w2[:, :, :])

            if cc == "rdma":
                # Allocate exchange state and pre-generate the remote-DMA
                # descriptors while stage-1 inputs stream in: descgen is
                # address-only (each prep defers its source read to the
                # trigger), and doing it here also pre-loads the
                # remote-dma gpsimd ucode library off the critical path.
                # The dummy collective (fabric init) also lives in this
                # critical section so its completion is not tracked by
                # Tile and cannot stall later critical-section entries.
                rsem = nc.alloc_semaphore("rsem")
                lsem = nc.alloc_semaphore("lsem")
                psem = nc.alloc_semaphore("psem")
                sb2 = s2pool.tile([128, NCORES, 4, BCHUNK], BF16, tag="sb2")
                rcv = s2pool.tile([128, NCORES, 4, BCHUNK], BF16, tag="rcv")
                dd_in = dpool.tile([4, 64], BF16, tag="ddin", name="ddin")
                dd_out = dpool.tile([64], BF16, tag="ddout", name="ddout")
                with tc.tile_critical(no_gpsimd_drain=True):
                    for dlt in range(1, NCORES):
                        rd = [None] * NCORES
                        rd[dlt] = (0, dlt)
                        nc.gpsimd.remote_dma_broadcast(
                            out_ap=rcv[:, dlt, :, :],
                            in_ap=sb2[:, dlt, :, :],
                            remote_sem=rsem,
                            local_sem=lsem,
                            rdests=rd,
                        ).then_inc(psem, 1)
                    nc.gpsimd.wait_ge(psem, NCORES - 1)

            # ---------------- stage 1: partial W.T @ flat.T ----------------
            acc = [
                psum1.tile([128, B], F32, tag=f"acc{t}", name=f"acc_{t}")
                for t in range(4)
            ]

            NKB = NKG * KB_PER_G  # 32 k-chunks of 128
            for kg in range(NKG):
                xg = xpool.tile([128, KB_PER_G * B], BF16, tag="xg")
                nc.sync.dma_start(out=xg[:], in_=memT[kg])
                w1t = wpool.tile([128, KB_PER_G * H], BF16, tag="w1t")
                nc.scalar.dma_start(out=w1t[:], in_=w1[kg])
                wgt = wpool.tile([128, KB_PER_G * H], BF16, tag="wgt")
                nc.scalar.dma_start(out=wgt[:], in_=wg[kg])
                for kb in range(KB_PER_G):
                    k = kg * KB_PER_G + kb
                    first = k == 0
                    last = k == NKB - 1
                    rhs = xg[:, bass.ts(kb, B)]
                    for h in range(2):
                        nc.tensor.matmul(
                            acc[h][:],
                            w1t[:, bass.ts(2 * kb + h, 128)],
                            rhs,
                            start=first,
                            stop=last,
                        )
                        nc.tensor.matmul(
                            acc[2 + h][:],
                            wgt[:, bass.ts(2 * kb + h, 128)],
                            rhs,
                            start=first,
                            stop=last,
                        )

            # PSUM -> SBUF (f32 -> bf16 downcast, bias/8 folded), laid out
            # [p, pos j, t, b] so each peer's slice is contiguous.
            if cc != "rdma":
                sb2 = s2pool.tile([128, NCORES, 4, BCHUNK], BF16, tag="sb2")
            for t in range(4):
                nc.vector.tensor_scalar_add(
                    sb2[:, :, t, :],
                    acc[t][:].rearrange("p (j b) -> p j b", j=NCORES),
                    bt[:, t : t + 1],
                )

            if cc == "rdma":
                # Hand-rolled reduce-scatter (see module docstring).  The
                # descriptors were pre-generated during stage 1; here we
                # only fire them once the sb2 copies are done, then wait
                # for the 7 peer slices to arrive.  The arrival wait lives
                # in a minimal tile_critical section because the
                # scheduling simulator cannot model remote semaphore
                # increments.
                nc.vector.tensor_copy(rcv[:, 0, :, :], sb2[:, 0, :, :])
                # Trigger + arrival wait in a minimal critical section:
                # its entry barrier orders the trigger after the sb2
                # copies, and the scheduler (which cannot model remote
                # semaphore increments) never sees the block.
                nc.gpsimd._pending_untriggered_insts[0] = []
                with tc.tile_critical(no_gpsimd_drain=True):
                    nc.gpsimd.trigger_dma(count=NCORES - 1)
                    nc.vector.wait_ge(rsem, 2 * (NCORES - 1))
                a1 = s2pool.tile([128, 4, 4, BCHUNK], BF16, tag="a1")
                nc.vector.tensor_add(a1[:], rcv[:, 0:4, :, :], rcv[:, 4:8, :, :])
                a2 = s2pool.tile([128, 2, 4, BCHUNK], BF16, tag="a2")
                nc.vector.tensor_add(a2[:], a1[:, 0:2, :, :], a1[:, 2:4, :, :])
                s2f = s2pool.tile([128, 4, BCHUNK], BF16, tag="s2in")
                nc.vector.tensor_add(s2f[:], a2[:, 0, :, :], a2[:, 1, :, :])
                s2a, s2b = s2f[:, 0:2, :], s2f[:, 2:4, :]
            else:
                # CC-stack fallback: DRAM staging + single ReduceScatter.
                ccin = dpool.tile(
                    [NCORES, 128, 4 * BCHUNK], BF16, tag="ccin", name="ccin"
                )
                for j in range(NCORES):
                    nc.gpsimd.dma_start(out=ccin[j], in_=sb2[:, j, :, :])
                ccout = dpool.tile([128, 4 * BCHUNK], BF16, tag="ccout", name="ccout")
                nc.gpsimd.collective_compute(
                    "ReduceScatter",
                    mybir.AluOpType.add,
                    replica_groups=GROUPS,
                    ins=[ccin[:].opt()],
                    outs=[ccout[:].opt()],
                )
                cct = ccout.rearrange("p (t b) -> p t b", t=4)
                s2a_t = s2pool.tile([128, 2, BCHUNK], BF16, tag="s2a")
                nc.sync.dma_start(out=s2a_t[:], in_=cct[:, 0:2, :])
                s2b_t = s2pool.tile([128, 2, BCHUNK], BF16, tag="s2b")
                nc.scalar.dma_start(out=s2b_t[:], in_=cct[:, 2:4, :])
                s2a, s2b = s2a_t[:], s2b_t[:]

            # ---------------- stage 2: gate + W2 ----------------
            _stage2(nc, s2pool, psum1, s2a, s2b, bt, w2t, outT)

            if cc == "rdma":
                # Fabric/routing init happens at NEFF load because this
                # collective exists in the program; placed at the end so
                # its CC-stream execution overlaps the kernel tail
                # instead of delaying the exchange.
                nc.gpsimd.collective_compute(
                    "ReduceScatter",
                    mybir.AluOpType.add,
                    replica_groups=[[0, 1, 2, 3], [4, 5, 6, 7]],
                    ins=[dd_in[:].opt()],
                    outs=[dd_out[:].opt()],
                )

    nc.compile()
    return nc


def _shard(x, memory, W1, b1, Wg, bg, W2, b2, cc=CC):
    """Build the 8 per-core input maps from the full problem inputs."""
    import ml_dtypes

    bf16 = ml_dtypes.bfloat16
    x = np.asarray(x, dtype=np.float32)
    memory = np.asarray(memory, dtype=np.float32)
    W1 = np.asarray(W1, dtype=np.float32)
    Wg = np.asarray(Wg, dtype=np.float32)
    W2 = np.ascontiguousarray(np.asarray(W2, dtype=np.float32))
    b1 = np.asarray(b1, dtype=np.float32)
    bg = np.asarray(bg, dtype=np.float32)
    b2 = np.asarray(b2, dtype=np.float32)

    # rolled ring buffer, flattened and transposed: [IN, B]
    flatT = np.empty((IN, B), dtype=np.float32)
    flatT[:V] = x.T
    flatT[V:] = memory[:, : M - 1, :].reshape(B, IN - V).T
    e = 1.0 / NCORES
    bpk = np.ascontiguousarray(
        np.stack(
            [b1[:128] * e, b1[128:] * e, bg[:128] * e, bg[128:] * e, b2[:128], b2[128:]],
            axis=1,
        )
    )
    w2p = np.ascontiguousarray(W2.reshape(2, 128, V).transpose(1, 0, 2).astype(bf16))

    def pack(A):
        # [KC, X] -> [NKG, 128, KB_PER_G * X]: per-partition lines are
        # KB_PER_G consecutive k-chunks' rows, contiguous in DRAM.
        X = A.shape[1]
        return np.ascontiguousarray(
            A.reshape(NKG, KB_PER_G, 128, X)
            .transpose(0, 2, 1, 3)
            .reshape(NKG, 128, KB_PER_G * X)
        )

    in_maps = []
    for c in range(NCORES):
        sl = slice(KC * c, KC * (c + 1))
        slab = flatT[sl]
        if cc == "rdma":
            # Column-block j must hold the batch chunk of the core whose
            # PHYSICAL id is phys(c)^j (remote_dma_broadcast XORs physical
            # ids).  phys is the driver's virtual->physical NC map, probed
            # once on this platform (self-inverse: 4<->6, 5<->7).
            phys = [0, 1, 2, 3, 6, 7, 4, 5]
            perm = np.concatenate(
                [
                    np.arange(BCHUNK) + BCHUNK * phys[phys[c] ^ j]
                    for j in range(NCORES)
                ]
            )
            slab = slab[:, perm]
        in_maps.append(
            {
                "memT": pack(slab.astype(bf16)),
                "w1": pack(W1[sl].astype(bf16)),
                "wg": pack(Wg[sl].astype(bf16)),
                "w2": w2p,
                "bpk": bpk,
            }
        )
    return in_maps


def _get_nc():
    if "nc" not in _CACHE:
        _CACHE["nc"] = _build()
    return _CACHE["nc"]


def kernel(x, memory, W1, b1, Wg, bg, W2, b2, **run_kwargs):
    nc = _get_nc()
    in_maps = _shard(x, memory, W1, b1, Wg, bg, W2, b2)
    res = bass_utils.run_bass_kernel_spmd(
        nc, in_maps, core_ids=list(range(NCORES)), **run_kwargs
    )
    _CACHE["last_results"] = res
    out = np.empty((B, V), dtype=np.float32)
    for c in range(NCORES):
        out[c * BCHUNK : (c + 1) * BCHUNK, :] = res.results[c]["outT"].T.astype(
            np.float32
        )
    return out
